# revision 2
# baseline (speedup 1.0000x reference)
"""Trainium2 Bass kernel for a single-layer transformer encoder block, v2.

Data-parallel over batch (8 cores x 1 sequence). Per-core redesign around
fp8 DoubleRow matmuls (2 stacked K-tiles per instruction at 0.5 cyc/row =
4x fp32r MAC throughput at M=128):

  LN1 (DVE) -> h bf16 -> bf16 PE transposes -> hT e4m3
  -> q/k/v projections as DR-fp8; q/k psum column order [A0 B0 A1 B1] so
  two plain [64,T] copies produce the [32,2,T] per-head layout scores-DR
  needs -> scores DR (K=2x32, head B at base partition 32) -> exp on ACT
  (1024 cols/instr, e4m3 out, ts-pairs packed as o's DR rhs) -> o DR with
  stationary vext = [v(64)|ones|0 pad] M=96, softmax denominator lands in
  psum row 64 -> normalize (DVE reciprocal + gpsimd partition_broadcast +
  DVE mul) -> oT e4m3 -> proj DR + residual + LN2 (fp32 h2)
  -> fp32 PE transposes -> h2T hi(e4)/lo(e5m2)
  -> FFN: weights host-scaled into e4m3's normal range (32*W1, 64*W2) and
  split hi(e4)/lo(e5m2); activations h2T/u split hi/lo too; each GEMM runs
  3 DR streams (hi*hi + lo*hi + hi*lo) into one psum group. Descale by
  1/32 inside the ACT relu and 1/64 in the FFN2 psum copy.

Attention is ACT(exp)-bound; FFN is PE-bound. The token range runs as 4
quarters with FFN1(Q-1)/FFN2(Q-2) interleaved into attention(Q)'s
instruction stream so PE and ACT overlap.
"""

import sys

for _p in ("/opt/trn_rl_repo", "/root/.axon_site/_ro/trn_rl_repo"):
    if _p not in sys.path:
        sys.path.append(_p)

import numpy as np
import ml_dtypes

import concourse.bass as bass
import concourse.bacc as bacc
import concourse.mybir as mybir
import concourse.tile as tile
from concourse import masks
from concourse import library_config
from concourse.bass_utils import run_bass_kernel_spmd

F32 = mybir.dt.float32
BF16 = mybir.dt.bfloat16
E4 = mybir.dt.float8e4
E5 = mybir.dt.float8e5
AF = mybir.ActivationFunctionType
ALU = mybir.AluOpType
PM = mybir.MatmulPerfMode
NP_E4 = ml_dtypes.float8_e4m3
NP_E5 = ml_dtypes.float8_e5m2

B = 8
T = 1024
C = 768
H = 12
HS = 64
F = 3072
EPS = 1e-5
SCALE = 1.0 / float(np.sqrt(C))
W1S = 32.0  # host scale on W1 (keeps fp8 out of subnormals)
W2S = 64.0  # host scale on W2

NT = T // 128
KC = C // 128
KP = 3
NQ = 4
FB = F // 128
FPAIR = 12

DEFAULT_FLAGS = {
    "g1_one": False, "be1_zero": False, "g2_one": False, "be2_zero": False,
    "bq_zero": False, "bk_zero": False, "bv_zero": False, "bp_zero": False,
    "b1_zero": False, "b2_zero": False,
}


def _bcast_ap(dram_ap, parts=128):
    return bass.AP(
        tensor=dram_ap.tensor,
        offset=dram_ap.offset,
        ap=[[0, parts]] + [list(d) for d in dram_ap.ap],
    )


def _perpart_ap(dram_ap, cols):
    return bass.AP(
        tensor=dram_ap.tensor,
        offset=dram_ap.offset,
        ap=[[1, 128], [128, cols]],
    )


def split_excess_waits(nc, max_waits=1):
    """This walrus build rejects instructions carrying more than one sem
    wait. Move excess waits onto dedicated NoOps."""
    for f in nc.m.functions:
        for bb in f.blocks:
            insts = list(bb.instructions)
            out = []
            changed = False
            for inst in insts:
                si = inst.sync_info
                if si is not None and si.on_wait and len(si.on_wait) > max_waits:
                    waits = list(si.on_wait)
                    extra, keep = waits[:-max_waits], waits[-max_waits:]
                    for i in range(0, len(extra), max_waits):
                        nop = mybir.InstNoOp(name=f"I-waitsplit-{nc.next_id()}")
                        nop.engine = inst.engine
                        nop.sync_info = mybir.SyncInfo(
                            on_wait=extra[i : i + max_waits], on_update=[]
                        )
                        out.append(nop)
                    inst.sync_info = mybir.SyncInfo(
                        on_wait=keep, on_update=list(si.on_update)
                    )
                    changed = True
                out.append(inst)
            if changed:
                bb.instructions[:] = out


def build_kernel(split_waits=True, flags=None):
    fl = dict(DEFAULT_FLAGS)
    if flags:
        fl.update(flags)

    nc = bacc.Bacc()

    x_d = nc.dram_tensor("x", [T, C], F32, kind="ExternalInput")
    wqk_d = nc.dram_tensor("wqk8", [128, 2 * KC * KP * 2 * 128], E4, kind="ExternalInput")
    wv_d = nc.dram_tensor("wv8", [128, KP * 2 * C], E4, kind="ExternalInput")
    wp_d = nc.dram_tensor("wp8", [128, KP * 2 * C], E4, kind="ExternalInput")
    w1hi_d = nc.dram_tensor("w1hi8", [128, KP * 2 * F], E4, kind="ExternalInput")
    w1lo_d = nc.dram_tensor("w1lo8", [128, KP * 2 * F], E5, kind="ExternalInput")
    w2hi_d = nc.dram_tensor("w2hi8", [128, FPAIR * 2 * C], E4, kind="ExternalInput")
    w2lo_d = nc.dram_tensor("w2lo8", [128, FPAIR * 2 * C], E5, kind="ExternalInput")
    bq_d = nc.dram_tensor("bq", [H, HS], F32, kind="ExternalInput")
    bk_d = nc.dram_tensor("bk", [H, HS], F32, kind="ExternalInput")
    bv_d = nc.dram_tensor("bv", [H, HS], F32, kind="ExternalInput")
    bp_d = nc.dram_tensor("bp", [C], F32, kind="ExternalInput")
    b1_d = nc.dram_tensor("b1", [F], F32, kind="ExternalInput")
    b2_d = nc.dram_tensor("b2", [C], F32, kind="ExternalInput")
    g1_d = nc.dram_tensor("g1", [C], F32, kind="ExternalInput")
    be1_d = nc.dram_tensor("beta1", [C], F32, kind="ExternalInput")
    g2_d = nc.dram_tensor("g2", [C], F32, kind="ExternalInput")
    be2_d = nc.dram_tensor("beta2", [C], F32, kind="ExternalInput")
    out_d = nc.dram_tensor("out", [T, C], F32, kind="ExternalOutput")

    with tile.TileContext(nc) as tc:
        consts = tc.alloc_tile_pool(name="consts", bufs=1)
        ident = consts.tile([128, 128], BF16, name="ident")
        masks.make_identity(nc, ident[:])
        nc.gpsimd.load_library(library_config.attn)
        eps_t = consts.tile([128, 1], F32, name="eps_t")
        nc.vector.memset(eps_t[:], EPS)

        def bcast_const(name, dram_ap, skip):
            if skip:
                return None
            t = consts.tile([128, C], F32, name=name)
            nc.sync.dma_start(out=t[:], in_=_bcast_ap(dram_ap))
            return t

        g1b = bcast_const("g1b", g1_d[:], fl["g1_one"])
        be1b = bcast_const("be1b", be1_d[:], fl["be1_zero"])
        g2b = bcast_const("g2b", g2_d[:], fl["g2_one"])
        be2b = bcast_const("be2b", be2_d[:], fl["be2_zero"])
        bpb = bcast_const("bpb", bp_d[:], fl["bp_zero"])
        b2b = bcast_const("b2b", b2_d[:], fl["b2_zero"])
        bvb = bcast_const("bvb", bv_d[:, :].rearrange("h d -> (h d)"), fl["bv_zero"])
        b1_sb = None
        if not fl["b1_zero"]:
            b1_sb = consts.tile([128, FB], F32, name="b1_sb")
            nc.sync.dma_start(out=b1_sb[:], in_=_perpart_ap(b1_d[:], FB))
            # psum carries 32*(h2@W1); bias must be pre-scaled to match
            nc.vector.tensor_scalar_mul(out=b1_sb[:], in0=b1_sb[:], scalar1=W1S)

        # ---------------- persistent SBUF ----------------
        bigA = tc.alloc_tile_pool(name="bigA", bufs=1)
        h_t = [bigA.tile([128, C], BF16, name=f"h_{i}") for i in range(NT)]
        h2_t = [bigA.tile([128, C], BF16, name=f"h2_{i}") for i in range(NT)]
        q2 = [bigA.tile([64, 2, T], E4, name=f"q2_{co}") for co in range(KC)]
        k2 = [bigA.tile([64, 2, T], E4, name=f"k2_{co}") for co in range(KC)]
        vext = bigA.tile([128, 4, H, 2, 96], E4, name="vext")
        oT = bigA.tile([128, KC, T], E4, name="oT")
        h2T_hi = bigA.tile([128, KC, T], E4, name="h2T_hi")
        h2T_lo = bigA.tile([128, KC, T], E5, name="h2T_lo")
        wp_sb = bigA.tile([128, KP, 2, C], E4, name="wp_sb")
        w1hi = bigA.tile([128, KP, 2, F], E4, name="w1hi")
        w1lo = bigA.tile([128, KP, 2, F], E5, name="w1lo")
        w2hi = bigA.tile([128, FPAIR, 2, C], E4, name="w2hi")
        w2lo = bigA.tile([128, FPAIR, 2, C], E5, name="w2lo")

        work = tc.alloc_tile_pool(name="work", bufs=1)

        # setup-lifetime tensors (released before FFN rings are allocated)
        poolX = tc.alloc_tile_pool(name="poolX", bufs=1)
        hT = poolX.tile([128, KC, T], E4, name="hT")
        wq_sb = poolX.tile([128, KC, KP, 2, 128], E4, name="wq_sb")
        wk_sb = poolX.tile([128, KC, KP, 2, 128], E4, name="wk_sb")
        poolWV = tc.alloc_tile_pool(name="poolWV", bufs=1)
        wv_sb = poolWV.tile([128, KP, 2, C], E4, name="wv_sb")


        ps_pre = tc.alloc_tile_pool(name="ps_pre", bufs=1, space="PSUM")

        # ---------------- LN helper (stats DVE, apply DVE) ----------------
        def layernorm(src_tile, dst_tile, gb, bb, sfx):
            stats = work.tile([128, 3, 6], F32, name=f"stats{sfx}", tag="stats", bufs=2)
            for g in range(3):
                nc.vector.bn_stats(
                    out=stats[:, g, :], in_=src_tile[:, g * 256 : (g + 1) * 256]
                )
            mv = work.tile([128, 2], F32, name=f"mv{sfx}", tag="mv", bufs=2)
            nc.vector.bn_aggr(out=mv[:], in_=stats[:])
            rstd = work.tile([128, 1], F32, name=f"rstd{sfx}", tag="rstd", bufs=2)
            nc.scalar.activation(
                out=rstd[:], in_=mv[:, 1:2], func=AF.Sqrt, bias=eps_t[:]
            )
            nc.vector.reciprocal(out=rstd[:], in_=rstd[:])
            nc.vector.tensor_scalar(
                out=dst_tile[:],
                in0=src_tile[:],
                scalar1=mv[:, 0:1],
                scalar2=rstd[:],
                op0=ALU.subtract,
                op1=ALU.mult,
            )
            if gb is not None:
                nc.vector.tensor_mul(out=dst_tile[:], in0=dst_tile[:], in1=gb[:])
            if bb is not None:
                nc.vector.tensor_add(out=dst_tile[:], in0=dst_tile[:], in1=bb[:])

        # ---------------- phase 0: x -> LN1 -> h -> hT ----------------
        for i in range(NT):
            xt = work.tile([128, C], F32, name="xt", tag="xy", bufs=2)
            nc.sync.dma_start(out=xt[:], in_=x_d[i * 128 : (i + 1) * 128, :])
            layernorm(xt, h_t[i], g1b, be1b, "1")

        for i in range(NT):
            for g in range(2):
                pst = ps_pre.tile([128, 3, 128], BF16, name="pst", tag="tr", bufs=2)
                for jj in range(3):
                    j = g * 3 + jj
                    nc.tensor.transpose(
                        pst[:, jj, :], h_t[i][:, j * 128 : (j + 1) * 128], ident[:]
                    )
                nc.scalar.activation(
                    out=hT[:, g * 3 : g * 3 + 3, i * 128 : (i + 1) * 128],
                    in_=pst[:], func=AF.Copy,
                )


        # weight DMAs, emitted after phase 0 so x tiles win the SP queue;
        # ordered by first consumption (v -> qk -> proj -> ffn)
        wv_flat = wv_d[:, :].rearrange("p (kp x) -> p kp x", kp=KP)
        for kp in range(KP):
            nc.sync.dma_start(
                out=wv_sb[:, kp, :, :].rearrange("p a b -> p (a b)"),
                in_=wv_flat[:, kp, :],
            )
        wqk_flat = wqk_d[:, :].rearrange("p (a co x) -> p a co x", a=2, co=KC)
        for co in range(KC):
            nc.sync.dma_start(
                out=wq_sb[:, co, :, :, :].rearrange("p a b c -> p (a b c)"),
                in_=wqk_flat[:, 0, co, :],
            )
            nc.sync.dma_start(
                out=wk_sb[:, co, :, :, :].rearrange("p a b c -> p (a b c)"),
                in_=wqk_flat[:, 1, co, :],
            )
        wp_flat = wp_d[:, :].rearrange("p (kp x) -> p kp x", kp=KP)
        for kp in range(KP):
            nc.sync.dma_start(
                out=wp_sb[:, kp, :, :].rearrange("p a b -> p (a b)"),
                in_=wp_flat[:, kp, :],
            )
        for dst, srcd in ((w1hi, w1hi_d), (w1lo, w1lo_d)):
            fl_ap = srcd[:, :].rearrange("p (kp i x) -> p kp i x", kp=KP, i=2)
            for kp in range(KP):
                for i in range(2):
                    nc.sync.dma_start(
                        out=dst[:, kp, i, :], in_=fl_ap[:, kp, i, :]
                    )
        for dst, srcd in ((w2hi, w2hi_d), (w2lo, w2lo_d)):
            fl_ap = srcd[:, :].rearrange("p (c6 x) -> p c6 x", c6=6)
            for c6 in range(6):
                nc.sync.dma_start(
                    out=dst[:, 2 * c6 : 2 * c6 + 2, :, :].rearrange(
                        "p a b c -> p (a b c)"
                    ),
                    in_=fl_ap[:, c6, :],
                )

        # ---------------- v projection ----------------
        nc.gpsimd.memset(vext[:, :, :, :, 64:96], 0.0)
        nc.vector.memset(vext[:, :, :, :, 64:65], 1.0)

        for i in range(NT):
            pv = ps_pre.tile([128, C], F32, name="pv", tag="pv", bufs=1)
            for n in range(3):
                for kp in range(KP):
                    nc.tensor.matmul(
                        pv[:, n * 256 : (n + 1) * 256],
                        hT[:, 2 * kp : 2 * kp + 2, i * 128 : (i + 1) * 128],
                        wv_sb[:, kp, :, n * 256 : (n + 1) * 256],
                        start=(kp == 0),
                        stop=(kp == KP - 1),
                        perf_mode=PM.DoubleRow,
                    )
            if bvb is not None:
                nc.vector.tensor_add(out=pv[:], in0=pv[:], in1=bvb[:])
            nc.scalar.activation(
                out=vext[:, i // 2, :, i % 2, 0:64],
                in_=pv[:].rearrange("p (h d) -> p h d", d=64),
                func=AF.Copy,
            )

        # ---------------- q/k projection ----------------
        def qk_block(co, pq_of):
            for si, (nm, w_sb, dst, eng) in enumerate((
                ("q", wq_sb, q2, "act"),
                ("k", wk_sb, k2, "dve"),
            )):
                pq = pq_of(si)
                for tch in range(4):
                    for kp in range(KP):
                        nc.tensor.matmul(
                            pq[:, tch * 256 : (tch + 1) * 256],
                            w_sb[:, co, kp, :, :],
                            hT[:, 2 * kp : 2 * kp + 2, tch * 256 : (tch + 1) * 256],
                            start=(kp == 0),
                            stop=(kp == KP - 1),
                            perf_mode=PM.DoubleRow,
                        )
                if not fl[f"b{nm}_zero"]:
                    bflat = (bq_d if nm == "q" else bk_d)[:, :].rearrange(
                        "h d -> (h d)"
                    )
                    bqp = work.tile([128, 1], F32, name=f"b{nm}p", tag="bqp", bufs=2)
                    for half in range(2):
                        for hh in range(2):
                            nc.sync.dma_start(
                                out=bqp[
                                    half * 64 + hh * 32 : half * 64 + hh * 32 + 32, :
                                ],
                                in_=bass.AP(
                                    tensor=bflat.tensor,
                                    offset=bflat.offset + (2 * co + hh) * HS + half * 32,
                                    ap=[[1, 32], [1, 1]],
                                ),
                            )
                    nc.vector.tensor_scalar_add(out=pq[:], in0=pq[:], scalar1=bqp[:])
                for half in range(2):
                    src = pq[half * 64 : (half + 1) * 64, :]
                    if eng == "act":
                        nc.scalar.activation(
                            out=dst[co][:, half, :], in_=src, func=AF.Copy
                        )
                    else:
                        nc.vector.tensor_copy(out=dst[co][:, half, :], in_=src)

        poolWV.release()
        ps_pre.release()

        # ---------------- FFN activation rings (2 quarters deep) ----------
        poolY = tc.alloc_tile_pool(name="poolY", bufs=1, side="right")
        uhi = poolY.tile([128, FB, 2, 256], E4, name="uhi")
        ulo = poolY.tile([128, FB, 2, 256], E5, name="ulo")

        eab_pool = tc.alloc_tile_pool(name="eab_pool", bufs=1, side="right")
        small = tc.alloc_tile_pool(name="small", bufs=1, side="right")

        ps = tc.alloc_tile_pool(name="ps", bufs=1, space="PSUM")
        sc_ps = [ps.tile([128, 1024], F32, name=f"sc{i}", tag=f"sc{i}") for i in range(2)]
        qk_block(0, lambda si: sc_ps[si][:])
        o_ps = ps.tile([96, 4, 256], F32, name="o_ps", tag="o_ps")
        f1_ps = ps.tile([128, 2, 256], F32, name="f1_ps", tag="f1_ps")
        f1b_ps = f1_ps
        mm_ps = ps.tile([128, 2, 256], F32, name="mm_ps", tag="mm_ps")

        def emit_proj_tt(tt):
            yt = work.tile([128, C], F32, name="yt", tag="xy", bufs=2)
            for n in range(3):
                pr = mm_ps[:, n % 2, :]
                for kp in range(KP):
                    nc.tensor.matmul(
                        pr,
                        oT[:, 2 * kp : 2 * kp + 2, tt * 128 : (tt + 1) * 128],
                        wp_sb[:, kp, :, n * 256 : (n + 1) * 256],
                        start=(kp == 0),
                        stop=(kp == KP - 1),
                        perf_mode=PM.DoubleRow,
                    )
                nc.vector.tensor_add(
                    out=yt[:, n * 256 : (n + 1) * 256],
                    in0=pr,
                    in1=h_t[tt][:, n * 256 : (n + 1) * 256],
                )
            if bpb is not None:
                nc.vector.tensor_add(out=yt[:], in0=yt[:], in1=bpb[:])
            layernorm(yt, h2_t[tt], g2b, be2b, "2")

        def emit_h2T_tt(tt):
            # borrow scores psum slots; transpose bf16 via bitcast view
            for g in range(2):
                psv3 = (
                    sc_ps[g][:, 0:192].bitcast(BF16)
                    .rearrange("p (a b) -> p a b", a=3)
                )
                for jj in range(3):
                    j = g * 3 + jj
                    nc.tensor.transpose(
                        psv3[:, jj, :],
                        h2_t[tt][:, j * 128 : (j + 1) * 128],
                        ident[:],
                    )
                hi_dst = h2T_hi[:, g * 3 : g * 3 + 3, tt * 128 : (tt + 1) * 128]
                nc.scalar.activation(out=hi_dst, in_=psv3[:], func=AF.Copy)
                nc.vector.tensor_sub(
                    out=h2T_lo[:, g * 3 : g * 3 + 3, tt * 128 : (tt + 1) * 128],
                    in0=psv3[:],
                    in1=hi_dst,
                )

        def emit_ffn1_fill(q, fill):
            pt = f1_ps if fill % 2 == 0 else f1b_ps
            slot = q % 2
            for sub in range(2):
                fb = 2 * fill + sub
                streams = (
                    (w1hi, h2T_hi), (w1hi, h2T_lo), (w1lo, h2T_hi),
                )
                for si, (w, hh2) in enumerate(streams):
                    for kp in range(KP):
                        nc.tensor.matmul(
                            pt[:, sub, :],
                            w[:, kp, :, fb * 128 : (fb + 1) * 128],
                            hh2[:, 2 * kp : 2 * kp + 2, q * 256 : (q + 1) * 256],
                            start=(si == 0 and kp == 0),
                            stop=(si == 2 and kp == KP - 1),
                            perf_mode=PM.DoubleRow,
                        )
            fb0 = 2 * fill
            if b1_sb is not None:
                for sub in range(2):
                    nc.vector.tensor_scalar_add(
                        out=pt[:, sub, :], in0=pt[:, sub, :],
                        scalar1=b1_sb[:, fb0 + sub : fb0 + sub + 1],
                    )
            wbf = work.tile([128, 2, 256], BF16, name="wbf", tag="wbf", bufs=2)
            nc.vector.tensor_scalar(
                out=wbf[:], in0=pt[:], scalar1=1.0 / W1S, scalar2=0.0,
                op0=ALU.mult, op1=ALU.max,
            )
            hi_dst = uhi[:, fb0 : fb0 + 2, slot, :]
            nc.gpsimd.tensor_copy(out=hi_dst, in_=wbf[:])
            nc.gpsimd.tensor_sub(
                out=ulo[:, fb0 : fb0 + 2, slot, :], in0=wbf[:], in1=hi_dst
            )

        def emit_ffn2_fill(q, fill):
            tt = 2 * q + fill // 3
            n = fill % 3
            slot = q % 2
            tl = fill // 3
            pr = mm_ps[:, n % 2, :]
            streams = ((uhi, w2hi), (ulo, w2hi), (uhi, w2lo))
            first = True
            for si, (uu, w) in enumerate(streams):
                for fp in range(FPAIR):
                    nc.tensor.matmul(
                        pr,
                        uu[:, 2 * fp : 2 * fp + 2, slot, tl * 128 : (tl + 1) * 128],
                        w[:, fp, :, n * 256 : (n + 1) * 256],
                        start=first,
                        stop=(si == 2 and fp == FPAIR - 1),
                        perf_mode=PM.DoubleRow,
                    )
                    first = False
            yo1 = work.tile([128, 256], F32, name="yo1", tag="yo1", bufs=1)
            nc.vector.tensor_scalar_mul(out=yo1[:], in0=pr, scalar1=1.0 / W2S)
            yoc = work.tile([128, 256], F32, name="yoc", tag="yoc", bufs=1)
            nc.gpsimd.tensor_add(
                out=yoc[:], in0=yo1[:], in1=h2_t[tt][:, n * 256 : (n + 1) * 256]
            )
            if b2b is not None:
                nc.vector.tensor_add(
                    out=yoc[:], in0=yoc[:], in1=b2b[:, n * 256 : (n + 1) * 256]
                )
            nc.sync.dma_start(
                out=out_d[tt * 128 : (tt + 1) * 128, n * 256 : (n + 1) * 256],
                in_=yoc[:],
            )

        def pipe_tasks(q):
            tasks = []
            if q >= 1:
                prev = q - 1
                tasks.append(lambda tt=2 * prev: emit_proj_tt(tt))
                tasks.append(lambda tt=2 * prev: emit_h2T_tt(tt))
                tasks.append(lambda tt=2 * prev + 1: emit_proj_tt(tt))
                tasks.append(lambda tt=2 * prev + 1: emit_h2T_tt(tt))
                for fill in range(12):
                    tasks.append(lambda p=prev, f=fill: emit_ffn1_fill(p, f))
            if q >= 2:
                pprev = q - 2
                for fill in range(6):
                    tasks.append(lambda p=pprev, f=fill: emit_ffn2_fill(p, f))
            return tasks

        for q in range(NQ):
            if q == 1:
                poolX.release()
            tasks = pipe_tasks(q)
            ntask = len(tasks)
            unit = 0
            for jp in range(KC):
                for hh in range(2):
                    e_t = eab_pool.tile(
                        [128, 4, 2, 256], E4, name="eab", tag="eab", bufs=2
                    )
                    for filli in range(2):
                        pst = sc_ps[filli]
                        for tsl in range(4):
                            ts = filli * 4 + tsl
                            nc.tensor.matmul(
                                pst[:, tsl * 256 : (tsl + 1) * 256],
                                k2[jp][hh * 32 : hh * 32 + 32, :,
                                       ts * 128 : (ts + 1) * 128],
                                q2[jp][hh * 32 : hh * 32 + 32, :,
                                       q * 256 : (q + 1) * 256],
                                start=True,
                                stop=True,
                                perf_mode=PM.DoubleRow,
                            )
                        nc.scalar.activation(
                            out=e_t[:, 2 * filli : 2 * filli + 2, :, :],
                            in_=pst[:],
                            func=AF.Exp,
                            scale=SCALE,
                        )
                    hgl = 2 * jp + hh
                    opar = (2 * jp + hh) % 4
                    for sp in range(4):
                        nc.tensor.matmul(
                            o_ps[:, opar, :],
                            vext[:, sp, hgl, :, :],
                            e_t[:, sp, :, :],
                            start=(sp == 0),
                            stop=(sp == 3),
                            perf_mode=PM.DoubleRow,
                        )
                    rec = small.tile([1, 256], F32, name="rec", tag="rec", bufs=1)
                    nc.vector.reciprocal(out=rec[:], in_=o_ps[64:65, opar, :])
                    bcast = small.tile([64, 256], F32, name="bcast", tag="bc", bufs=1)
                    nc.gpsimd.partition_broadcast(bcast[:], rec[:])
                    nc.vector.tensor_mul(
                        out=oT[hh * 64 : hh * 64 + 64, jp, q * 256 : (q + 1) * 256],
                        in0=o_ps[0:64, opar, :],
                        in1=bcast[:],
                    )
                    if q == 0 and hh == 1 and jp + 1 < KC:
                        qk_block(jp + 1, lambda si: sc_ps[si][:])
                    hi = (unit + 1) * ntask // 12
                    lo = unit * ntask // 12
                    for t in range(lo, hi):
                        tasks[t]()
                    unit += 1

        # ---------------- tail ----------------
        q = NQ - 1
        tail = []
        tail.append(lambda: emit_proj_tt(2 * q))
        tail.append(lambda: emit_h2T_tt(2 * q))
        tail.append(lambda: emit_proj_tt(2 * q + 1))
        tail.append(lambda: emit_h2T_tt(2 * q + 1))
        for fill in range(12):
            tail.append(lambda f=fill: emit_ffn1_fill(q, f))
        for fill in range(6):
            tail.append(lambda f=fill: emit_ffn2_fill(q - 1, f))
        for fill in range(6):
            tail.append(lambda f=fill: emit_ffn2_fill(q, f))
        for t in tail:
            t()

        ps.release()
        small.release()
        eab_pool.release()
        poolY.release()
        work.release()
        bigA.release()
        consts.release()

    if split_waits:
        nc.finalize()
        split_excess_waits(nc)
    return nc


def prep_weights(inputs):
    f32 = np.float32
    Wq = np.asarray(inputs["Wq"], f32)
    Wk = np.asarray(inputs["Wk"], f32)
    Wv = np.asarray(inputs["Wv"], f32)
    Wp = np.asarray(inputs["Wp"], f32)
    W1 = np.asarray(inputs["W1"], f32)
    W2 = np.asarray(inputs["W2"], f32)

    def pack_qk(W):
        outp = np.empty((128, KC, KP, 2, 128), f32)
        for co in range(KC):
            a, b = W[2 * co], W[2 * co + 1]
            cols = np.concatenate(
                [a[:, 0:32], b[:, 0:32], a[:, 32:64], b[:, 32:64]], axis=1
            )
            outp[:, co] = cols.reshape(KP, 2, 128, 128).transpose(2, 0, 1, 3)
        return outp.astype(NP_E4)

    wq8 = pack_qk(Wq)
    wk8 = pack_qk(Wk)
    wqk8 = np.stack(
        [wq8.reshape(128, -1), wk8.reshape(128, -1)], axis=1
    ).reshape(128, -1)

    def pack_cmajor(W, dt):  # [C, N] -> [128, KP, 2, N]
        N = W.shape[1]
        return W.reshape(KP, 2, 128, N).transpose(2, 0, 1, 3).astype(dt)

    Wv_flat = Wv.transpose(1, 0, 2).reshape(C, C)
    wv8 = pack_cmajor(Wv_flat, NP_E4).reshape(128, -1)
    wp8 = pack_cmajor(Wp, NP_E4).reshape(128, -1)

    W1s = W1S * W1
    w1hi_f = W1s.astype(NP_E4).astype(f32)
    w1hi = pack_cmajor(w1hi_f, NP_E4).reshape(128, -1)
    w1lo = pack_cmajor(W1s - w1hi_f, NP_E5).reshape(128, -1)

    def pack_fmajor(W, dt):  # [F, C] -> [128, FPAIR, 2, C]
        return W.reshape(FPAIR, 2, 128, C).transpose(2, 0, 1, 3).astype(dt)

    W2s = W2S * W2
    w2hi_f = W2s.astype(NP_E4).astype(f32)
    w2hi = pack_fmajor(w2hi_f, NP_E4).reshape(128, -1)
    w2lo = pack_fmajor(W2s - w2hi_f, NP_E5).reshape(128, -1)

    return {
        "wqk8": np.ascontiguousarray(wqk8),
        "wv8": np.ascontiguousarray(wv8),
        "wp8": np.ascontiguousarray(wp8),
        "w1hi8": np.ascontiguousarray(w1hi),
        "w1lo8": np.ascontiguousarray(w1lo),
        "w2hi8": np.ascontiguousarray(w2hi),
        "w2lo8": np.ascontiguousarray(w2lo),
    }


def input_flags(inputs):
    def allzero(a):
        return bool(np.all(np.asarray(a) == 0.0))

    def allone(a):
        return bool(np.all(np.asarray(a) == 1.0))

    return {
        "g1_one": allone(inputs["g1"]),
        "be1_zero": allzero(inputs["beta1"]),
        "g2_one": allone(inputs["g2"]),
        "be2_zero": allzero(inputs["beta2"]),
        "bq_zero": allzero(inputs["bq"]),
        "bk_zero": allzero(inputs["bk"]),
        "bv_zero": allzero(inputs["bv"]),
        "bp_zero": allzero(inputs["bp"]),
        "b1_zero": allzero(inputs["b1"]),
        "b2_zero": allzero(inputs["b2"]),
    }


def kernel(**inputs):
    x = np.asarray(inputs["x"], dtype=np.float32)
    assert x.shape == (B, T, C), x.shape
    shared = prep_weights(inputs)
    for name in ("bq", "bk", "bv", "bp", "b1", "b2", "g1", "beta1", "g2", "beta2"):
        shared[name] = np.ascontiguousarray(np.asarray(inputs[name], dtype=np.float32))

    nc = build_kernel(flags=input_flags(inputs))
    in_maps = [{"x": np.ascontiguousarray(x[b]), **shared} for b in range(B)]
    res = run_bass_kernel_spmd(nc, in_maps, list(range(B)))
    out = np.stack([res.results[b]["out"] for b in range(B)], axis=0)
    return out
